# revision 1
# baseline (speedup 1.0000x reference)
"""AttnBlock (GroupNorm -> QKV 1x1 -> full NxN attention -> proj -> residual)
for Trainium2, SPMD over 8 NeuronCores.

Sharding: data-parallel over batch (2) x query-pixel blocks (4 of 1024 px).
Each core receives its batch image x [C, N] PERMUTED so that its own query
block occupies pixels [0, NQ): attention is permutation-invariant over keys,
so K/V may be computed in any pixel order as long as it is consistent.  K and
V^T are computed redundantly per batch pair, queries are disjoint.  No
collectives.

Key structural points:
  - x and the QKV weights arrive as bf16; x is loaded ONCE into SBUF (plus
    an fp8 copy via gpsimd cast-DMA) and stays resident for stats, matmuls
    and the residual.  HW rel err 7.9e-3 vs the 2e-2 gate.
  - GroupNorm affine hn = A*x + B folds into the weights: fp8 production
    weights w8 = round_e4m3((8*A) * w_bf16) (x8 scale dodges e4m3
    subnormals; every PSUM drain descales by 1/8 via the activation scale).
    B-terms: K bias cancels in softmax; Q bias tq = Wq@B + bq via tiny bf16
    matvecs -> ACT bias; V bias commutes into the proj bias
    bp_dev = Wp@(Wv@B) + bpT.  No per-pixel hn is ever materialized.
  - ALL heavy matmuls run fp8 e4m3 in DoubleRow perf mode (2 contraction
    tiles per instruction, 0.5 cyc/row): K/V/Q production (s-subtile pairs
    of w8/x8), S^T = K^T@Q (K/Q stored fp8 at natural scale), and
    O += V^T-pair^T @ P-pair (V^T stored pair-interleaved [kt/2, 2, C]).
    Only the proj stays f32r for accuracy.
  - exp(s - 2.5) on ACT emits fp8 prob pairs (the shift keeps exp inside
    e4m3 range and cancels in the softmax ratio).  Softmax denominators are
    ALSO a DoubleRow matmul: an all-ones fp8 lhsT accumulates [128, QP]
    per-query sums across pairs, so the reciprocal is already partition-
    broadcast -- no Pool/DVE accumulator chains, no broadcast matmul.
  - proj runs on UNNORMALIZED O (division commutes through the linear
    proj); epilogue divides, adds bias + residual (one scalar_tensor_tensor
    per channel subtile), per-subtile out DMAs on alternating queues.
  - GroupNorm stats are engine-split: DVE bn_stats on 6 windows, ACT
    Identity/Square-with-accum on 2 (the earliest-arriving window goes to
    ACT, which is otherwise idle from t~2).
  - S + exp are emitted INTERLEAVED with production (chunk ch emits the
    S/exp pairs of chunk ch-2 for both query passes): the ACT exp stream
    fills drain gaps and ends early; all K/V drains go to DVE so the ACT
    queue is exp-dominated (re-adding drains to ACT measurably hurts:
    FIFO head-of-line blocking delays the last exp).  The wq8/wk8 A-folds
    partially run on ACT via activation scale=A8 (per-partition AP) in its
    idle windows.  Prob pairs for both passes live in SBUF, so the final
    AV/denominator loops are pure PE work.  PSUM: production 4 +
    matvec 1 + S-pipeline 3 banks in phase 2; O accumulators 4 +
    denom/proj 2 in the attention epilogue.
"""

from contextlib import ExitStack

import numpy as np

import concourse.bacc as bacc
import concourse.bass as bass
import concourse.mybir as mybir
import concourse.tile as tile

F32 = mybir.dt.float32
F32R = mybir.dt.float32r
BF16 = mybir.dt.bfloat16
FP8 = mybir.dt.float8e4
AF = mybir.ActivationFunctionType


def build_program(C=512, G=32, N=4096, NQ=1024, eps=1e-5, precision="tf32"):
    """Emit the per-core Bass program (SPMD; per-core data differs only)."""
    P = 128
    CS = C // P                  # channel subtiles
    KT = N // P                  # key/pixel tiles
    NCH = 512                    # phase-2 production chunk / bn window (px)
    NCHUNKS = N // NCH
    QCHUNKS = NQ // NCH          # leading chunks that are also query pixels
    NW = 1024                    # x-load quarter width
    NWQ = N // NW
    QP = min(512, NQ)            # query-pass width
    QPASSES = NQ // QP
    cpg = C // G                 # channels per group
    GPS = P // cpg               # groups per channel-subtile
    assert C % P == 0 and N % P == 0 and NQ % QP == 0 and P % cpg == 0
    assert NQ % NCH == 0 and N % NW == 0
    MMDT = F32R if precision == "tf32" else F32
    SDT = BF16                   # storage dtype for x / w / K / V^T / Q / P

    nc = bacc.Bacc(None, target_bir_lowering=False)

    x_d = nc.dram_tensor("x", [C, N], SDT, kind="ExternalInput")
    wt_d = {
        w: nc.dram_tensor(f"w{w}t", [C, C], SDT, kind="ExternalInput")
        for w in ("q", "k", "v")
    }
    wpt_d = nc.dram_tensor("wpt", [C, C], MMDT, kind="ExternalInput")
    bqT_d = nc.dram_tensor("bqT", [P, CS], F32, kind="ExternalInput")
    bpT_d = nc.dram_tensor("bpT", [P, CS], F32, kind="ExternalInput")
    gamma_d = nc.dram_tensor("gamma", [C], F32, kind="ExternalInput")
    beta_d = nc.dram_tensor("beta", [C], F32, kind="ExternalInput")
    indg_d = nc.dram_tensor("indg", [P, GPS], F32, kind="ExternalInput")
    inde_d = nc.dram_tensor("inde", [GPS, P], F32, kind="ExternalInput")
    out_d = nc.dram_tensor("out", [C, NQ], F32, kind="ExternalOutput")

    x_r = x_d[:, :].rearrange("(s p) n -> p s n", p=P)
    out_r = out_d[:, :].rearrange("(s p) n -> p s n", p=P)

    with tile.TileContext(nc) as tc, ExitStack() as st:
        const = st.enter_context(tc.tile_pool(name="const", bufs=1))
        big = st.enter_context(tc.tile_pool(name="big", bufs=1))
        small = st.enter_context(tc.tile_pool(name="small", bufs=1))
        ptp = st.enter_context(tc.tile_pool(name="ptp", bufs=2 * (4096 // 128) // 2))

        # resident big tensors
        x_sb = big.tile([P, CS, N], SDT, tag="x")        # x, loaded once
        x8 = big.tile([P, CS, N], FP8, tag="x8")         # fp8 production copy
        K_sb = big.tile([P, CS, N], FP8, tag="K")        # K[co, n]
        VT_sb = big.tile([P, KT // 2, 2, C], FP8, tag="VT")  # V^T pairs
        Q_sb = big.tile([P, CS, NQ], FP8, tag="Q")       # Q[co, nq] (scaled)
        wpT = big.tile([P, CS, C], MMDT, tag="wpT")      # proj weight

        # ---- x first: stats are the critical path -------------------------
        dma_engs = [nc.sync, nc.gpsimd]
        for qd in range(NCHUNKS):
            dma_engs[qd % len(dma_engs)].dma_start(
                out=x_sb[:, :, qd * NCH:(qd + 1) * NCH],
                in_=x_r[:, :, qd * NCH:(qd + 1) * NCH])
        for qd in range(NCHUNKS):       # fp8 copy for production matmuls
            nc.gpsimd.dma_start(
                out=x8[:, :, qd * NCH:(qd + 1) * NCH],
                in_=x_r[:, :, qd * NCH:(qd + 1) * NCH])

        # ---- weights + constants on the scalar queue ----------------------
        with ExitStack() as st1:
            wqkv = st1.enter_context(tc.tile_pool(name="wqkv", bufs=3))
            w8p = st1.enter_context(tc.tile_pool(name="w8p", bufs=3))
            ps_a = st1.enter_context(tc.tile_pool(name="ps_a", bufs=4,
                                                  space="PSUM"))
            ps_mv = st1.enter_context(tc.tile_pool(name="ps_mv", bufs=1,
                                                   space="PSUM"))
            ps_s = st1.enter_context(tc.tile_pool(name="ps_s", bufs=3,
                                                  space="PSUM"))

            gammaT = const.tile([P, CS], F32, tag="gammaT")
            nc.sync.dma_start(out=gammaT,
                              in_=gamma_d[:].rearrange("(s p) -> p s", p=P))
            betaT = const.tile([P, CS], F32, tag="betaT")
            nc.sync.dma_start(out=betaT,
                              in_=beta_d[:].rearrange("(s p) -> p s", p=P))
            indg = const.tile([P, GPS], F32, tag="indg")
            nc.sync.dma_start(out=indg, in_=indg_d[:, :])
            inde = const.tile([GPS, P], F32, tag="inde")
            nc.sync.dma_start(out=inde, in_=inde_d[:, :])
            wk = wqkv.tile([P, CS, C], SDT, tag="wt", name="w_k")
            nc.sync.dma_start(
                out=wk, in_=wt_d["k"][:, :].rearrange("(s p) c -> p s c", p=P))
            wv = wqkv.tile([P, CS, C], SDT, tag="wt", name="w_v")
            nc.sync.dma_start(
                out=wv, in_=wt_d["v"][:, :].rearrange("(s p) c -> p s c", p=P))
            bqT = const.tile([P, CS], F32, tag="bqT")
            nc.sync.dma_start(out=bqT, in_=bqT_d[:, :])
            bpT = const.tile([P, CS], F32, tag="bpT")
            nc.sync.dma_start(out=bpT, in_=bpT_d[:, :])
            wq = wqkv.tile([P, CS, C], SDT, tag="wt", name="w_q")
            nc.sync.dma_start(
                out=wq, in_=wt_d["q"][:, :].rearrange("(s p) c -> p s c", p=P))
            nc.sync.dma_start(
                out=wpT, in_=wpt_d[:, :].rearrange("(s p) c -> p s c", p=P))

            ones_c = const.tile([P, 1], F32, tag="ones_c")
            nc.vector.memset(ones_c, 1.0)
            ones_r = const.tile([1, P], F32, tag="ones_r")
            nc.vector.memset(ones_r, 1.0)
            eps_t = const.tile([P, 1], F32, tag="eps")
            nc.vector.memset(eps_t, eps)
            nc0_t = const.tile([P, 1], F32, tag="nc0")   # exp shift (fp8 range)
            nc.vector.memset(nc0_t, -2.5)
            ones8 = const.tile([P, 2, P], FP8, tag="ones8")  # denom lhsT
            nc.vector.memset(ones8, 1.0)
            ones_w = const.tile([P, NCH], SDT, tag="ones_w")
            nc.gpsimd.memset(ones_w, 1.0)
            # ACT table preload off the critical path (Sqrt: stats tail;
            # Exp: first attention tile)
            dume = small.tile([P, 1], F32, tag="dume")
            nc.scalar.activation(out=dume, in_=eps_t, func=AF.Sqrt)
            nc.scalar.activation(out=dume, in_=eps_t, func=AF.Exp)

            # ---- phase 1: GroupNorm stats over resident x -----------------
            # Three-way engine split (DVE bn_stats alone is the serial
            # bottleneck): DVE bn_stats on 5 windows; ACT Identity/Square
            # with accum_out on 1; Pool scalar_tensor_tensor accum on 2.
            WIN_DVE = [1, 2, 3, 4, 6, 7]
            WIN_ACT = [0, 5]
            WIN_POOL = []
            nA = len(WIN_DVE) * NCH
            stats_all = small.tile([P, CS, len(WIN_DVE), 6], F32, tag="stats")
            sxa = small.tile([P, CS, len(WIN_ACT), 2], F32, tag="sxa")
            sxp = (small.tile([P, CS, len(WIN_POOL), 2], F32, tag="sxp")
                   if WIN_POOL else None)
            scr = small.tile([P, 2, NCH], SDT, tag="scr")     # throwaway outs
            for w0 in range(NCHUNKS):
                x_w = lambda s: x_sb[:, s, w0 * NCH:(w0 + 1) * NCH]
                if w0 in WIN_DVE:
                    wi = WIN_DVE.index(w0)
                    for s in range(CS):
                        nc.vector.bn_stats(out=stats_all[:, s, wi, :],
                                           in_=x_w(s))
                elif w0 in WIN_ACT:
                    wi = WIN_ACT.index(w0)
                    for s in range(CS):
                        nc.scalar.activation(out=scr[:, 0, :], in_=x_w(s),
                                             func=AF.Identity,
                                             accum_out=sxa[:, s, wi, 0:1])
                        nc.scalar.activation(out=scr[:, 0, :], in_=x_w(s),
                                             func=AF.Square,
                                             accum_out=sxa[:, s, wi, 1:2])
                else:
                    wi = WIN_POOL.index(w0)
                    for s in range(CS):
                        nc.gpsimd.scalar_tensor_tensor(
                            out=scr[:, 1, :], in0=x_w(s), scalar=1.0,
                            in1=ones_w, op0=mybir.AluOpType.mult,
                            op1=mybir.AluOpType.mult,
                            accum_out=sxp[:, s, wi, 0:1])
                        nc.gpsimd.scalar_tensor_tensor(
                            out=scr[:, 1, :], in0=x_w(s), scalar=1.0,
                            in1=x_w(s), op0=mybir.AluOpType.mult,
                            op1=mybir.AluOpType.mult,
                            accum_out=sxp[:, s, wi, 1:2])
            mv = small.tile([P, CS, 2], F32, tag="mv")
            for s in range(CS):
                nc.vector.bn_aggr(out=mv[:, s, :], in_=stats_all[:, s, :, :])

            # per-channel [sum(x), sum(x^2)] -> group reduce via indicator
            # matmul.  DVE part: sum = mean*nA, sumsq = (var + mean^2)*nA.
            # the sum(x) and sum(x^2) halves are independent: DVE does the
            # first, Pool the second, halving the serial combine latency
            rhs8 = small.tile([P, 2 * CS], F32, tag="rhs8")
            nc.vector.tensor_scalar_mul(rhs8[:, 0:CS], mv[:, :, 0], float(nA))
            nc.gpsimd.tensor_mul(out=rhs8[:, CS:], in0=mv[:, :, 0], in1=mv[:, :, 0])
            nc.gpsimd.tensor_add(out=rhs8[:, CS:], in0=rhs8[:, CS:], in1=mv[:, :, 1])
            nc.gpsimd.tensor_scalar_mul(rhs8[:, CS:], rhs8[:, CS:], float(nA))
            for wi in range(len(WIN_ACT)):
                nc.vector.tensor_add(out=rhs8[:, 0:CS], in0=rhs8[:, 0:CS],
                                     in1=sxa[:, :, wi, 0])
                nc.gpsimd.tensor_add(out=rhs8[:, CS:], in0=rhs8[:, CS:],
                                     in1=sxa[:, :, wi, 1])
            for wi in range(len(WIN_POOL)):
                nc.vector.tensor_add(out=rhs8[:, 0:CS], in0=rhs8[:, 0:CS],
                                     in1=sxp[:, :, wi, 0])
                nc.gpsimd.tensor_add(out=rhs8[:, CS:], in0=rhs8[:, CS:],
                                     in1=sxp[:, :, wi, 1])
            ps_g = ps_a.tile([GPS, 2 * CS], F32, tag="abank")
            nc.tensor.matmul(ps_g, lhsT=indg, rhs=rhs8, start=True, stop=True)
            gtmp = small.tile([GPS, 2 * CS], F32, tag="gtmp")
            nc.vector.tensor_scalar_mul(gtmp, ps_g, 1.0 / (cpg * N))
            # gvar = E[x^2] - mean^2 ; grstd = 1/sqrt(gvar + eps)
            gsq = small.tile([GPS, CS], F32, tag="gsq")
            nc.vector.tensor_mul(out=gsq, in0=gtmp[:, 0:CS], in1=gtmp[:, 0:CS])
            e8 = small.tile([GPS, 2 * CS], F32, tag="e8")
            nc.vector.tensor_sub(out=e8[:, 0:CS], in0=gtmp[:, CS:], in1=gsq)
            nc.scalar.activation(out=e8[:, 0:CS], in_=e8[:, 0:CS], func=AF.Sqrt,
                                 bias=eps_t[:GPS], scale=1.0)
            nc.vector.reciprocal(out=e8[:, 0:CS], in_=e8[:, 0:CS])
            nc.vector.tensor_copy(out=e8[:, CS:], in_=gtmp[:, 0:CS])
            # expand groups -> channels
            ps_e = ps_a.tile([P, 2 * CS], F32, tag="abank")
            nc.tensor.matmul(ps_e, lhsT=inde, rhs=e8, start=True, stop=True)
            A_sb = small.tile([P, CS], F32, tag="A")     # A = gamma * rstd
            nc.vector.tensor_mul(out=A_sb, in0=ps_e[:, 0:CS], in1=gammaT)
            B_sb = small.tile([P, CS], SDT, tag="B")     # B = beta - A*mean
            nc.vector.tensor_mul(out=B_sb, in0=ps_e[:, CS:], in1=A_sb)
            nc.vector.tensor_sub(out=B_sb, in0=betaT, in1=B_sb)

            # ---- phase 2: K / V^T / Q from resident x (weights A-folded) --
            tq_sb = small.tile([P, CS], F32, tag="tq")
            u_sb = small.tile([P, CS], F32, tag="u")
            bp_dev = small.tile([P, CS], F32, tag="bp")

            # fp8 production weights: w8 = (8*A) * w_bf16, rounded to e4m3.
            # The x8 factor dodges e4m3 subnormals; every drain descales by
            # 1/8 so all stored tensors stay at natural scale.
            A8 = small.tile([P, CS], F32, tag="A8")
            nc.vector.tensor_scalar_mul(A8, A_sb, 8.0)
            wk8 = w8p.tile([P, CS, C], FP8, tag="w8", name="wk8")
            wv8 = w8p.tile([P, CS, C], FP8, tag="w8", name="wv8")
            wq8 = w8p.tile([P, CS, C], FP8, tag="w8", name="wq8")

            # S + exp interleaved with production: exp fills the ACT drain
            # gaps; fp8 prob pairs land in SBUF so the later AV loops are
            # pure PE work with no ACT dependency.
            pt_tiles = {qp_: {} for qp_ in range(QPASSES)}

            def emit_s_pair(qp_, pair):
                q0_ = qp_ * QP
                pt = ptp.tile([P, 2, QP], FP8, tag="pt",
                              name=f"pt_{qp_}_{pair}")
                pt_tiles[qp_][pair] = pt
                for half in range(2):
                    kt = 2 * pair + half
                    s_ps = ps_s.tile([P, QP], F32, tag="sbank",
                                     name=f"s_ps_{qp_}_{kt}")
                    for t in range(CS // 2):
                        nc.tensor.matmul(
                            s_ps,
                            lhsT=K_sb[:, 2 * t:2 * t + 2,
                                      kt * P:(kt + 1) * P],
                            rhs=Q_sb[:, 2 * t:2 * t + 2, q0_:q0_ + QP],
                            start=(t == 0), stop=(t == CS // 2 - 1),
                            perf_mode=mybir.MatmulPerfMode.DoubleRow,
                        )
                    nc.scalar.activation(out=pt[:, half, :], in_=s_ps,
                                         func=AF.Exp, bias=nc0_t, scale=1.0)

            for ch in range(NCHUNKS):
                c0 = ch * NCH
                if ch == 0:
                    for s in range(CS):
                        if s < 2:
                            nc.vector.tensor_scalar_mul(
                                wk8[:, s, :], wk[:, s, :], A8[:, s:s + 1])
                        else:
                            nc.scalar.activation(
                                out=wk8[:, s, :], in_=wk[:, s, :],
                                func=AF.Identity, scale=A8[:, s:s + 1])
                for cs in range(CS):     # K rows [co-sub, chunk] (bias drops)
                    ps_k = ps_a.tile([P, NCH], F32, tag="abank")
                    for t in range(CS // 2):
                        nc.tensor.matmul(
                            ps_k,
                            lhsT=wk8[:, 2 * t:2 * t + 2, cs * P:(cs + 1) * P],
                            rhs=x8[:, 2 * t:2 * t + 2, c0:c0 + NCH],
                            start=(t == 0), stop=(t == CS // 2 - 1),
                            perf_mode=mybir.MatmulPerfMode.DoubleRow,
                        )
                    nc.vector.tensor_scalar_mul(
                        K_sb[:, cs, c0:c0 + NCH], ps_k, 0.125)
                if ch == 0:
                    # tq = Wq_s @ B + bq_s (RAW wq: A-scaling comes after)
                    ps_t = ps_mv.tile([P, CS], F32, tag="mv", name="ps_tq")
                    for cs in range(CS):
                        for s in range(CS):
                            nc.tensor.matmul(
                                ps_t[:, cs:cs + 1],
                                lhsT=wq[:, s, cs * P:(cs + 1) * P],
                                rhs=B_sb[:, s:s + 1],
                                start=(s == 0), stop=(s == CS - 1),
                                skip_group_check=True,
                            )
                    nc.vector.tensor_add(out=tq_sb, in0=ps_t, in1=bqT)
                    # u = Wv @ B (RAW wv)
                    ps_u = ps_mv.tile([P, CS], F32, tag="mv", name="ps_u")
                    for cs in range(CS):
                        for s in range(CS):
                            nc.tensor.matmul(
                                ps_u[:, cs:cs + 1],
                                lhsT=wv[:, s, cs * P:(cs + 1) * P],
                                rhs=B_sb[:, s:s + 1],
                                start=(s == 0), stop=(s == CS - 1),
                                skip_group_check=True,
                            )
                    nc.vector.tensor_copy(out=u_sb, in_=ps_u)
                    # fp8 production copies of wv/wq (A8-folded)
                    for s in range(CS):
                        nc.gpsimd.tensor_scalar_mul(
                            wv8[:, s, :], wv[:, s, :], A8[:, s:s + 1])
                    for s in range(CS):
                        nc.scalar.activation(
                            out=wq8[:, s, :], in_=wq[:, s, :],
                            func=AF.Identity, scale=A8[:, s:s + 1])
                for ns in range(NCH // P):   # V^T rows [pixel-sub, all co]
                    ps_v = ps_a.tile([P, C], F32, tag="abank")
                    for t in range(CS // 2):
                        nc.tensor.matmul(
                            ps_v,
                            lhsT=x8[:, 2 * t:2 * t + 2,
                                    c0 + ns * P:c0 + (ns + 1) * P],
                            rhs=wv8[:, 2 * t:2 * t + 2, :],
                            start=(t == 0), stop=(t == CS // 2 - 1),
                            perf_mode=mybir.MatmulPerfMode.DoubleRow,
                        )
                    vi = ch * (NCH // P) + ns
                    nc.vector.tensor_scalar_mul(
                        VT_sb[:, vi // 2, vi % 2, :], ps_v, 0.125)
                if ch < QCHUNKS:             # Q rows (own block = chunks 0..)
                    for cs in range(CS):
                        ps_q = ps_a.tile([P, NCH], F32, tag="abank")
                        for t in range(CS // 2):
                            nc.tensor.matmul(
                                ps_q,
                                lhsT=wq8[:, 2 * t:2 * t + 2,
                                         cs * P:(cs + 1) * P],
                                rhs=x8[:, 2 * t:2 * t + 2, c0:c0 + NCH],
                                start=(t == 0), stop=(t == CS // 2 - 1),
                                perf_mode=mybir.MatmulPerfMode.DoubleRow,
                            )
                        nc.scalar.activation(
                            out=Q_sb[:, cs, c0:c0 + NCH], in_=ps_q,
                            func=AF.Identity, bias=tq_sb[:, cs:cs + 1],
                            scale=0.125,
                        )
                if ch == 3:
                    # bp_dev = Wp @ u + bpT  (tv = Wv@B + bv commutes through
                    # the softmax average into the proj bias; bv part is in
                    # host bpT already)
                    ps_z = ps_mv.tile([P, CS], F32, tag="mv", name="ps_z")
                    for cs in range(CS):
                        for s in range(CS):
                            nc.tensor.matmul(
                                ps_z[:, cs:cs + 1],
                                lhsT=wpT.bitcast(F32)[:, s, cs * P:(cs + 1) * P],
                                rhs=u_sb[:, s:s + 1],
                                start=(s == 0), stop=(s == CS - 1),
                                skip_group_check=True,
                            )
                    nc.vector.tensor_add(out=bp_dev, in0=ps_z, in1=bpT)
                if ch >= 2:
                    cc = ch - 2
                    for qp_ in range(QPASSES):
                        for pair in (2 * cc, 2 * cc + 1):
                            emit_s_pair(qp_, pair)
            for qp_ in range(QPASSES):
                for cc in (NCHUNKS - 2, NCHUNKS - 1):
                    for pair in (2 * cc, 2 * cc + 1):
                        emit_s_pair(qp_, pair)

        # ---- phase 3: attention + proj + residual, per query pass ---------
        with ExitStack() as st2:
            ocq = st2.enter_context(tc.tile_pool(name="ocq", bufs=2))
            outp = st2.enter_context(tc.tile_pool(name="outp", bufs=2))
            sm2 = st2.enter_context(tc.tile_pool(name="sm2", bufs=2))
            ps_o = st2.enter_context(tc.tile_pool(name="ps_o", bufs=CS,
                                                  space="PSUM"))
            ps_den = st2.enter_context(tc.tile_pool(name="ps_den", bufs=2,
                                                    space="PSUM"))

            for qp in range(QPASSES):
                q0 = qp * QP
                o_ps = []
                for _cs in range(CS):
                    o_tile = ps_o.tile([P, QP], F32, tag="o",
                                       name=f"o_{qp}_{_cs}")
                    o_ps.append(o_tile)
                den_ps = ps_den.tile([P, QP], F32, tag="den",
                                     name=f"den_{qp}")
                # pure-PE attention: probs were exp'd during production
                for pair in range(KT // 2):
                    pt = pt_tiles[qp][pair]
                    last = pair == KT // 2 - 1
                    for cs in range(CS):  # O[c,q] += V^T-pair^T @ P-pair
                        nc.tensor.matmul(
                            o_ps[cs],
                            lhsT=VT_sb[:, pair, :, cs * P:(cs + 1) * P],
                            rhs=pt,
                            start=(pair == 0), stop=last,
                            perf_mode=mybir.MatmulPerfMode.DoubleRow,
                        )
                    # softmax denominators on the PE: all-ones lhsT gives
                    # the partition-broadcast per-query sums
                    nc.tensor.matmul(
                        den_ps, lhsT=ones8, rhs=pt,
                        start=(pair == 0), stop=last,
                        perf_mode=mybir.MatmulPerfMode.DoubleRow,
                    )
                rec_bc = sm2.tile([P, QP], F32, tag="recbc")
                nc.vector.reciprocal(out=rec_bc, in_=den_ps)
                # drain O unnormalized (softmax division commutes through the
                # linear proj: out = (Wp@O)/den + bp + x); split DVE/ACT
                oc = ocq.tile([P, CS, QP], MMDT, tag="ocq")
                for cs in range(CS):
                    if cs < 2:
                        nc.vector.tensor_copy(out=oc[:, cs, :], in_=o_ps[cs])
                    else:
                        nc.scalar.activation(out=oc[:, cs, :], in_=o_ps[cs],
                                             func=AF.Identity, scale=1.0)
                ot = outp.tile([P, CS, QP], F32, tag="ot")
                tt = outp.tile([P, CS, QP], F32, tag="tt")
                for cs in range(CS):          # proj rows [co-sub, qpass]
                    ps_pp = ps_den.tile([P, QP], F32, tag="den",
                                        name=f"pp_{qp}_{cs}")
                    for s in range(CS):
                        nc.tensor.matmul(
                            ps_pp, lhsT=wpT[:, s, cs * P:(cs + 1) * P],
                            rhs=oc[:, s, :],
                            start=(s == 0), stop=(s == CS - 1),
                        )
                    nc.vector.tensor_mul(out=tt[:, cs, :], in0=ps_pp,
                                         in1=rec_bc)
                    nc.vector.scalar_tensor_tensor(
                        out=ot[:, cs, :], in0=tt[:, cs, :],
                        scalar=bp_dev[:, cs:cs + 1],
                        in1=x_sb[:, cs, q0:q0 + QP],
                        op0=mybir.AluOpType.add, op1=mybir.AluOpType.add,
                    )
                    (nc.sync if cs % 2 == 0 else nc.scalar).dma_start(
                        out=out_r[:, cs, q0:q0 + QP], in_=ot[:, cs, :])

    nc.finalize()
    return nc


def make_consts(P=128, cpg=16):
    GPS = P // cpg
    indg = np.zeros((P, GPS), np.float32)
    for p in range(P):
        indg[p, p // cpg] = 1.0
    inde = indg.T.copy()
    return {
        "indg": indg,
        "inde": inde,
    }


_PROGRAM_CACHE = {}


def _get_program(C, G, N, NQ, precision="tf32"):
    key = (C, G, N, NQ, precision)
    if key not in _PROGRAM_CACHE:
        _PROGRAM_CACHE[key] = build_program(C=C, G=G, N=N, NQ=NQ,
                                            precision=precision)
    return _PROGRAM_CACHE[key]


def make_in_maps(x, gn_w, gn_b, q_w, q_b, k_w, k_b, v_w, v_b, proj_w, proj_b,
                 n_cores=8, G=32):
    """Shard full inputs into per-core input maps (biases folded on host).

    Per-core x is pixel-permuted so the core's query block is first; attention
    is permutation-invariant over keys so K/V stay consistent.  x and the
    QKV weights ship as bf16.
    """
    import ml_dtypes
    bf = ml_dtypes.bfloat16
    f = lambda a: np.ascontiguousarray(np.asarray(a, dtype=np.float32))
    x = f(x)
    b, c, h, w = x.shape
    n = h * w
    qblocks = n_cores // b
    nq = n // qblocks
    cs = c // 128
    scale = np.float32(c ** -0.5)
    xf = x.reshape(b, c, n)

    def to_pcs(v):                       # [C] -> [128, CS] (c = 128*s + p)
        return np.ascontiguousarray(np.asarray(v, np.float32).reshape(cs, 128).T)

    common = {
        "wqt": np.ascontiguousarray((f(q_w).T * scale).astype(bf)),
        "wkt": np.ascontiguousarray(f(k_w).T.astype(bf)),
        "wvt": np.ascontiguousarray(f(v_w).T.astype(bf)),
        "wpt": f(proj_w).T.copy(),
        "bqT": to_pcs(f(q_b) * scale),
        "bpT": to_pcs(f(proj_w) @ f(v_b) + f(proj_b)),
        "gamma": f(gn_w), "beta": f(gn_b),
        **make_consts(cpg=c // G),
    }
    in_maps = []
    for i in range(n_cores):
        bi, qi = divmod(i, qblocks)
        xb = xf[bi]
        qs, qe = qi * nq, (qi + 1) * nq
        xperm = np.concatenate([xb[:, qs:qe], xb[:, :qs], xb[:, qe:]], axis=1)
        in_maps.append({
            **common,
            "x": np.ascontiguousarray(xperm.astype(bf)),
        })
    return in_maps, (b, c, h, w, n, nq, qblocks)


def kernel(x, gn_w, gn_b, q_w, q_b, k_w, k_b, v_w, v_b, proj_w, proj_b):
    from concourse.bass_utils import run_bass_kernel_spmd

    in_maps, (b, c, h, w, n, nq, qblocks) = make_in_maps(
        x, gn_w, gn_b, q_w, q_b, k_w, k_b, v_w, v_b, proj_w, proj_b
    )
    n_cores = 8
    nc = _get_program(C=c, G=32, N=n, NQ=nq)
    res = run_bass_kernel_spmd(nc, in_maps, list(range(n_cores))).results
    out = np.empty((b, c, n), np.float32)
    for i in range(n_cores):
        bi, qi = divmod(i, qblocks)
        out[bi, :, qi * nq:(qi + 1) * nq] = res[i]["out"]
    return out.reshape(b, c, h, w)



# revision 29
# speedup vs baseline: 1.4057x; 1.4057x over previous
"""AttnBlock (GroupNorm -> QKV 1x1 -> full NxN attention -> proj -> residual)
for Trainium2, SPMD over 8 NeuronCores.

Sharding: data-parallel over batch (2) x query-pixel blocks (4 of 1024 px).
Each core receives its batch image x [C, N] PERMUTED so that its own query
block occupies pixels [0, NQ); attention is permutation-invariant over keys.
No collectives.

v3 structure — K and V are never materialized.  Weight products fold on the
host; GroupNorm folds into tiny per-channel vectors on device:

  hn = A*x + B (per-channel).  With P0 = s*Wk^T@Wq and W2 = Wp@Wv (host):
    S[k,q]  = x[:,k]^T @ QtA[:,q]   (+ per-q consts that cancel in softmax)
    QtA     = diag(A) (P0^T diag(A) x_q + P0^T B + s Wk^T bq)
    out     = W2 (A . Z0) / den + (W2 B + Wp bv + bp) + x,   Z0 = x @ P^T

  Device tensors: x bf16 (stats + residual), x8 fp8 [c,n] (S lhsT + query
  rhs), xt8 fp8 [k,c] pair-interleaved (Z lhsT; host-transposed), p0t bf16,
  w2t bf16, one packed const vector.  GroupNorm A enters via a [C,C] lhsT
  row-scale (P0A8 fold), a drain scale on Qt, and a drain scale on Z0; all
  B / bias terms ride along as drain biases or the proj bias.

  Heavy matmuls (Qt production, S, Z0, softmax denominators) run fp8 e4m3
  DoubleRow (0.5 cyc/row); proj runs bf16.

  Scheduling notes (cost-model driven):
  - All input DMAs ride the SP queue in priority order (p0t, x, x8, xt8,
    w2t); the DMA engines are a single ~25us serial resource, so order is
    everything.  Stats windows pipeline with the x chunks as they land.
  - Per 512-px window: DVE does the sums (tensor_scalar+accum runs 4x on
    bf16) + one sumsq; ACT two sumsq (Square+accum); Pool one sumsq.
  - ACT loads the Sqrt table set first (it also holds Identity/Square for
    the stats window), switches to the Exp set once, then owns the exp
    stream; every other drain lives on DVE/Pool.
  - S->exp->Z0 pipeline: S pairs on PE feed ACT exp; Z0/den DoubleRow
    matmuls trail one pair behind; the previous qpass's proj (and the
    bp_dev matvec) slot into PE gaps of the exp-bound stream.
"""

from contextlib import ExitStack

import numpy as np

import concourse.bacc as bacc
import concourse.bass as bass
import concourse.mybir as mybir
import concourse.tile as tile

F32 = mybir.dt.float32
F32R = mybir.dt.float32r
BF16 = mybir.dt.bfloat16
FP8 = mybir.dt.float8e4
AF = mybir.ActivationFunctionType
MUL = mybir.AluOpType.mult
ADD = mybir.AluOpType.add

SC_P0 = 64.0      # fp8 P0A lhsT pre-scale (dodges e4m3 subnormals)
SC_QT = 16.0      # fp8 Qt storage scale


def build_program(C=512, G=32, N=4096, NQ=1024, eps=1e-5, precision="tf32"):
    """Emit the per-core Bass program (SPMD; per-core data differs only)."""
    P = 128
    CS = C // P                  # channel subtiles
    KT = N // P                  # key/pixel tiles
    NCH = 512                    # x DMA chunk / stats window (px)
    NCHUNKS = N // NCH
    QP = min(512, NQ)            # query-pass width
    QPASSES = NQ // QP
    cpg = C // G                 # channels per group
    GPS = P // cpg               # groups per channel-subtile
    assert C % P == 0 and N % P == 0 and NQ % QP == 0 and P % cpg == 0
    SDT = BF16

    nc = bacc.Bacc(None, target_bir_lowering=False)

    x_d = nc.dram_tensor("x", [C, N], SDT, kind="ExternalInput")
    x8_d = nc.dram_tensor("x8", [C, N], FP8, kind="ExternalInput")
    xt8_d = nc.dram_tensor("xt8", [P, KT // 2, 2, C], FP8, kind="ExternalInput")
    p0t_d = nc.dram_tensor("p0t", [C, C], SDT, kind="ExternalInput")
    w2t_d = nc.dram_tensor("w2t", [C, C], SDT, kind="ExternalInput")
    CPW = 4 * CS + GPS + P
    cpk_d = nc.dram_tensor("cpk", [P, CPW], F32, kind="ExternalInput")
    out_d = nc.dram_tensor("out", [C, NQ], SDT, kind="ExternalOutput")

    x_r = x_d[:, :].rearrange("(s p) n -> p s n", p=P)
    x8_r = x8_d[:, :].rearrange("(s p) n -> p s n", p=P)
    p0_r = p0t_d[:, :].rearrange("(s p) o -> p s o", p=P)
    w2_r = w2t_d[:, :].rearrange("(s p) o -> p s o", p=P)
    out_r = out_d[:, :].rearrange("(s p) n -> p s n", p=P)

    with tile.TileContext(nc) as tc, ExitStack() as st:
        const = st.enter_context(tc.tile_pool(name="const", bufs=1))
        big = st.enter_context(tc.tile_pool(name="big", bufs=1))
        small = st.enter_context(tc.tile_pool(name="small", bufs=1))
        ptp = st.enter_context(tc.tile_pool(name="ptp", bufs=2 * (KT // 2)))

        # resident big tensors
        x_sb = big.tile([P, CS, N], SDT, tag="x")          # x bf16
        x8 = big.tile([P, CS, N], FP8, tag="x8")           # x fp8 [c, n]
        xt8 = big.tile([P, KT // 2, 2, C], FP8, tag="xt8")  # x^T fp8 pairs
        p0t = big.tile([P, CS, C], SDT, tag="p0t")         # s*Wk^T Wq (lhsT)
        p0a8 = big.tile([P, CS, C], FP8, tag="p0a8")       # A-folded fp8 P0
        w2t = big.tile([P, CS, C], SDT, tag="w2t")         # Wp@Wv (lhsT)
        qt8 = big.tile([P, CS, NQ], FP8, tag="qt8")        # QtA fp8
        cpk = const.tile([P, CPW], F32, tag="cpk")

        # ---- input DMAs: one queue (SP), priority order --------------------
        nc.sync.dma_start(out=cpk, in_=cpk_d[:, :])
        for qd in range(NCHUNKS):
            nc.sync.dma_start(out=x_sb[:, :, qd * NCH:(qd + 1) * NCH],
                              in_=x_r[:, :, qd * NCH:(qd + 1) * NCH])
        nc.sync.dma_start(out=p0t, in_=p0_r)
        for qd in range(4):
            nc.sync.dma_start(
                out=x8[:, :, qd * 1024:(qd + 1) * 1024],
                in_=x8_r[:, :, qd * 1024:(qd + 1) * 1024])
        for qd in range(4):
            nc.sync.dma_start(
                out=xt8[:, qd * (KT // 8):(qd + 1) * (KT // 8), :, :],
                in_=xt8_d[:, qd * (KT // 8):(qd + 1) * (KT // 8), :, :])
        nc.sync.dma_start(out=w2t, in_=w2_r)

        gammaT = cpk[:, 0:CS]
        betaT = cpk[:, CS:2 * CS]
        h0T = cpk[:, 2 * CS:3 * CS]
        bp2T = cpk[:, 3 * CS:4 * CS]
        indg = cpk[:, 4 * CS:4 * CS + GPS]
        inde = cpk[0:GPS, 4 * CS + GPS:4 * CS + GPS + P]

        with ExitStack() as st1:
            ps_sm = st1.enter_context(tc.tile_pool(name="ps_sm", bufs=2,
                                                   space="PSUM"))
            ps_qt = st1.enter_context(tc.tile_pool(name="ps_qt", bufs=2,
                                                   space="PSUM"))

            nc0_t = const.tile([P, 1], F32, tag="nc0")   # exp shift (fp8 rng)
            nc.vector.memset(nc0_t, -2.5)
            ones8 = const.tile([P, 2, P], FP8, tag="ones8")  # denom lhsT
            nc.vector.memset(ones8, 1.0)
            # single ACT table load for the whole kernel: the Exp set also
            # holds Identity/Square (stats + folds); rsqrt happens on DVE
            # via Newton, so Sqrt's set is never needed.  Loading now also
            # wins the DMA-engine queue before the big input transfers.
            dume = small.tile([P, 1], F32, tag="dume")
            nc.scalar.activation(out=dume, in_=nc0_t, func=AF.Exp)

            # ---- phase 1: GroupNorm stats, pipelined with the x DMAs ------
            # Pool cannot reduce (no accum) and tensor_tensor_reduce does
            # not exist on hw, so: DVE bn_stats on 6 windows, ACT Identity/
            # Square+accum on one late 1024-px double window.
            WIN_DVE = [0, 1, 2, 3, 6, 7]
            PX_ACT = (2048, 3072)
            nA = len(WIN_DVE) * NCH
            stats_all = small.tile([P, CS, len(WIN_DVE), 6], F32, tag="stats")
            sxa = small.tile([P, CS, 2], F32, tag="sxa")
            scr = small.tile([P, 2, 1024], SDT, tag="scr")
            for wi, w0 in enumerate(WIN_DVE):
                for s in range(CS):
                    nc.vector.bn_stats(
                        out=stats_all[:, s, wi, :],
                        in_=x_sb[:, s, w0 * NCH:(w0 + 1) * NCH])
            for s in range(CS):
                nc.scalar.activation(out=scr[:, 0, :],
                                     in_=x_sb[:, s, PX_ACT[0]:PX_ACT[1]],
                                     func=AF.Identity,
                                     accum_out=sxa[:, s, 0:1])
                nc.scalar.activation(out=scr[:, 0, :],
                                     in_=x_sb[:, s, PX_ACT[0]:PX_ACT[1]],
                                     func=AF.Square,
                                     accum_out=sxa[:, s, 1:2])
            mv = small.tile([P, CS, 2], F32, tag="mv")
            for s in range(CS):
                nc.vector.bn_aggr(out=mv[:, s, :], in_=stats_all[:, s, :, :])

            # combine: rhs8 = [sum(x) | sum(x^2)] per channel; DVE does the
            # sum half, Pool the sumsq half (SBUF-only, legal on Pool)
            rhs8 = small.tile([P, 2 * CS], F32, tag="rhs8")
            nc.vector.tensor_scalar_mul(rhs8[:, 0:CS], mv[:, :, 0], float(nA))
            nc.gpsimd.tensor_mul(out=rhs8[:, CS:], in0=mv[:, :, 0],
                                 in1=mv[:, :, 0])
            nc.gpsimd.tensor_add(out=rhs8[:, CS:], in0=rhs8[:, CS:],
                                 in1=mv[:, :, 1])
            nc.gpsimd.tensor_scalar_mul(rhs8[:, CS:], rhs8[:, CS:], float(nA))
            nc.vector.tensor_add(out=rhs8[:, 0:CS], in0=rhs8[:, 0:CS],
                                 in1=sxa[:, :, 0])
            nc.gpsimd.tensor_add(out=rhs8[:, CS:], in0=rhs8[:, CS:],
                                 in1=sxa[:, :, 1])
            ps_g = ps_sm.tile([GPS, 2 * CS], F32, tag="sm", name="ps_g")
            nc.tensor.matmul(ps_g, lhsT=indg, rhs=rhs8, start=True, stop=True)
            gtmp = small.tile([GPS, 2 * CS], F32, tag="gtmp")
            nc.vector.tensor_scalar_mul(gtmp, ps_g, 1.0 / (cpg * N))
            # gvar = E[x^2] - mean^2 ; grstd = 1/sqrt(gvar + eps)
            gsq = small.tile([GPS, CS], F32, tag="gsq")
            nc.vector.tensor_mul(out=gsq, in0=gtmp[:, 0:CS], in1=gtmp[:, 0:CS])
            e8 = small.tile([GPS, 2 * CS], F32, tag="e8")
            wv = small.tile([GPS, CS], F32, tag="wv")
            nc.vector.tensor_sub(out=wv, in0=gtmp[:, CS:], in1=gsq)
            nc.vector.tensor_scalar_add(wv, wv, eps)   # w = gvar + eps
            # rstd = rsqrt(w) by Newton on DVE (w ~ 1 for normalized input;
            # seed 1.5 - w/2 is the tangent at 1, two steps to fp32 noise)
            y_t = e8[:, 0:CS]
            nc.vector.tensor_scalar(out=y_t, in0=wv, scalar1=-0.5,
                                    scalar2=1.5, op0=MUL, op1=ADD)
            nwt = small.tile([GPS, CS], F32, tag="nwt")
            for _ in range(1):   # seed err ~4e-3 -> ~2e-5 after one step
                nc.vector.tensor_mul(out=nwt, in0=y_t, in1=y_t)
                nc.vector.tensor_mul(out=nwt, in0=nwt, in1=wv)
                nc.vector.tensor_scalar(out=nwt, in0=nwt, scalar1=-0.5,
                                        scalar2=1.5, op0=MUL, op1=ADD)
                nc.vector.tensor_mul(out=y_t, in0=y_t, in1=nwt)
            nc.vector.tensor_copy(out=e8[:, CS:], in_=gtmp[:, 0:CS])
            # expand groups -> channels
            ps_e = ps_sm.tile([P, 2 * CS], F32, tag="sm", name="ps_e")
            nc.tensor.matmul(ps_e, lhsT=inde, rhs=e8, start=True, stop=True)
            A_sb = small.tile([P, CS], F32, tag="A")     # A = gamma * rstd
            nc.vector.tensor_mul(out=A_sb, in0=ps_e[:, 0:CS], in1=gammaT)
            B32 = small.tile([P, CS], F32, tag="B32")    # B = beta - A*mean
            nc.vector.tensor_mul(out=B32, in0=ps_e[:, CS:], in1=A_sb)
            nc.vector.tensor_sub(out=B32, in0=betaT, in1=B32)
            B_sb = small.tile([P, CS], SDT, tag="B")
            nc.vector.tensor_copy(out=B_sb, in_=B32)

            # ---- phase 2: P0A fold, bias matvec, Qt production ------------
            # P0A8 = fp8(SC_P0 * A_c * P0); all 4 subtiles gate every Qt
            # matmul, so split the fold DVE/ACT (Identity is in the Exp set)
            a64 = small.tile([P, CS], F32, tag="a64")
            nc.vector.tensor_scalar_mul(a64, A_sb, SC_P0)
            for s in range(CS):
                if s < 2:
                    nc.vector.tensor_scalar_mul(
                        p0a8[:, s, :], p0t[:, s, :], a64[:, s:s + 1])
                else:
                    nc.scalar.activation(
                        out=p0a8[:, s, :], in_=p0t[:, s, :],
                        func=AF.Identity, scale=a64[:, s:s + 1])

            # r0 = P0^T B + h0 (Q-bias term of S, varies per key channel)
            ps_r = ps_sm.tile([P, CS], F32, tag="sm", name="ps_r")
            for cs in range(CS):
                for s in range(CS):
                    nc.tensor.matmul(
                        ps_r[:, cs:cs + 1],
                        lhsT=p0t[:, s, cs * P:(cs + 1) * P],
                        rhs=B_sb[:, s:s + 1],
                        start=(s == 0), stop=(s == CS - 1),
                        skip_group_check=True,
                    )
            qdr_s = small.tile([P, CS], F32, tag="qdr_s")
            nc.vector.tensor_scalar_mul(qdr_s, A_sb, SC_QT / SC_P0)
            qdr_b = small.tile([P, CS], F32, tag="qdr_b")
            nc.vector.tensor_add(out=qdr_b, in0=ps_r, in1=h0T)
            nc.vector.tensor_mul(out=qdr_b, in0=qdr_b, in1=A_sb)
            nc.vector.tensor_scalar_mul(qdr_b, qdr_b, SC_QT)

            # Qt production: QtA8 = fp8(SC_QT*A_o*(psum/SC_P0 + r0_o))
            for ch in range(NQ // 512):
                for cs in range(CS):
                    ps_q = ps_qt.tile([P, 512], F32, tag="qt")
                    for t in range(CS // 2):
                        nc.tensor.matmul(
                            ps_q,
                            lhsT=p0a8[:, 2 * t:2 * t + 2, cs * P:(cs + 1) * P],
                            rhs=x8[:, 2 * t:2 * t + 2,
                                   ch * 512:(ch + 1) * 512],
                            start=(t == 0), stop=(t == CS // 2 - 1),
                            perf_mode=mybir.MatmulPerfMode.DoubleRow,
                        )
                    # Pool cannot read PSUM on hw: drains stay on DVE
                    nc.vector.tensor_scalar(
                        out=qt8[:, cs, ch * 512:(ch + 1) * 512],
                        in0=ps_q,
                        scalar1=qdr_s[:, cs:cs + 1],
                        scalar2=qdr_b[:, cs:cs + 1],
                        op0=MUL, op1=ADD,
                    )

        # ---- phase 3: S -> exp -> Z0/den stream + proj + residual ---------
        with ExitStack() as st2:
            ocq = st2.enter_context(tc.tile_pool(name="ocq", bufs=2))
            outp = st2.enter_context(tc.tile_pool(name="outp", bufs=2))
            sm2 = st2.enter_context(tc.tile_pool(name="sm2", bufs=2))
            # 3 banks S stream (shared with proj psum) + 5 banks Z0/den
            ps_s = st2.enter_context(tc.tile_pool(name="ps_s", bufs=3,
                                                  space="PSUM"))
            ps_o = st2.enter_context(tc.tile_pool(name="ps_o", bufs=CS + 1,
                                                  space="PSUM"))

            bp_dev = small.tile([P, CS], F32, tag="bp")
            pt_tiles = {}

            def emit_s_pair(qp_, pair):
                q0_ = qp_ * QP
                pt = ptp.tile([P, 2, QP], FP8, tag="pt",
                              name=f"pt_{qp_}_{pair}")
                pt_tiles[(qp_, pair)] = pt
                for half in range(2):
                    kt = 2 * pair + half
                    s_ps = ps_s.tile([P, QP], F32, tag="sbank",
                                     name=f"s_ps_{qp_}_{kt}")
                    for t in range(CS // 2):
                        nc.tensor.matmul(
                            s_ps,
                            lhsT=x8[:, 2 * t:2 * t + 2, kt * P:(kt + 1) * P],
                            rhs=qt8[:, 2 * t:2 * t + 2, q0_:q0_ + QP],
                            start=(t == 0), stop=(t == CS // 2 - 1),
                            perf_mode=mybir.MatmulPerfMode.DoubleRow,
                        )
                    nc.scalar.activation(out=pt[:, half, :], in_=s_ps,
                                         func=AF.Exp, bias=nc0_t,
                                         scale=1.0 / SC_QT)

            def emit_z_pair(qp_, pair, o_ps, den_ps):
                pt = pt_tiles[(qp_, pair)]
                last = pair == KT // 2 - 1
                for cs in range(CS):
                    nc.tensor.matmul(
                        o_ps[cs],
                        lhsT=xt8[:, pair, :, cs * P:(cs + 1) * P],
                        rhs=pt,
                        start=(pair == 0), stop=last,
                        perf_mode=mybir.MatmulPerfMode.DoubleRow,
                    )
                nc.tensor.matmul(
                    den_ps, lhsT=ones8, rhs=pt,
                    start=(pair == 0), stop=last,
                    perf_mode=mybir.MatmulPerfMode.DoubleRow,
                )

            def emit_bp_matvec():
                # bp_dev = W2 @ B + (Wp bv + bp); w2t lands late, so this
                # slots into the qp0 stream well after the fold
                ps_z = ps_s.tile([P, CS], F32, tag="sbank", name="ps_z")
                for cs in range(CS):
                    for s in range(CS):
                        nc.tensor.matmul(
                            ps_z[:, cs:cs + 1],
                            lhsT=w2t[:, s, cs * P:(cs + 1) * P],
                            rhs=B_sb[:, s:s + 1],
                            start=(s == 0), stop=(s == CS - 1),
                            skip_group_check=True,
                        )
                nc.vector.tensor_add(out=bp_dev, in0=ps_z, in1=bp2T)

            def emit_proj_cs(qp_, cs, oc, rec_bc, ot, tt):
                q0_ = qp_ * QP
                ps_pp = ps_s.tile([P, QP], F32, tag="sbank",
                                  name=f"pp_{qp_}_{cs}")
                for s in range(CS):
                    nc.tensor.matmul(
                        ps_pp, lhsT=w2t[:, s, cs * P:(cs + 1) * P],
                        rhs=oc[:, s, :],
                        start=(s == 0), stop=(s == CS - 1),
                    )
                # tt reads PSUM -> DVE; the bias+residual adds are SBUF-only
                # and ride Pool (scalar_tensor_tensor is illegal there, so
                # two ops: +bias via AP scalar, then +x)
                nc.vector.tensor_mul(out=tt[:, cs, :], in0=ps_pp, in1=rec_bc)
                nc.gpsimd.tensor_scalar_add(ot[:, cs, :], tt[:, cs, :],
                                            bp_dev[:, cs:cs + 1])
                nc.gpsimd.tensor_add(out=ot[:, cs, :], in0=ot[:, cs, :],
                                     in1=x_sb[:, cs, q0_:q0_ + QP])
                # out rides SP only: a waiting dma_start holds its engine's
                # SEQ, and ACT/Pool must keep streaming
                nc.sync.dma_start(
                    out=out_r[:, cs, q0_:q0_ + QP], in_=ot[:, cs, :])

            # Z trails S: 3 pairs at first (rides out the late xt8 DMA
            # arrival for qp0 without head-blocking PE), catching back up to
            # a 1-pair lag so the post-stream tail stays short
            prev = None   # (oc, rec_bc, ot, tt) of the previous qpass
            for qp in range(QPASSES):
                o_ps = [ps_o.tile([P, QP], F32, tag="o", name=f"o_{qp}_{c}")
                        for c in range(CS)]
                den_ps = ps_o.tile([P, QP], F32, tag="o", name=f"den_{qp}")
                zdone = 0
                for pair in range(KT // 2):
                    emit_s_pair(qp, pair)
                    zlag = 3 if (qp == 0 and pair < 6) else 1
                    while zdone <= pair - zlag:
                        emit_z_pair(qp, zdone, o_ps, den_ps)
                        zdone += 1
                    if qp == 0 and pair == 10:
                        emit_bp_matvec()
                    if prev is not None and pair in (1, 2, 3, 4):
                        emit_proj_cs(qp - 1, pair - 1, *prev)
                while zdone < KT // 2:
                    emit_z_pair(qp, zdone, o_ps, den_ps)
                    zdone += 1
                rec_bc = sm2.tile([P, QP], F32, tag="recbc", name=f"rb_{qp}")
                nc.vector.reciprocal(out=rec_bc, in_=den_ps)
                # drain Z0 with the GroupNorm A fold (division by den
                # commutes through the linear proj)
                oc = ocq.tile([P, CS, QP], SDT, tag="ocq")
                for cs in range(CS):
                    nc.vector.tensor_scalar_mul(oc[:, cs, :], o_ps[cs],
                                                A_sb[:, cs:cs + 1])
                ot = outp.tile([P, CS, QP], SDT, tag="ot")
                tt = outp.tile([P, CS, QP], F32, tag="tt")
                prev = (oc, rec_bc, ot, tt)
            for cs in range(CS):
                emit_proj_cs(QPASSES - 1, cs, *prev)

    nc.finalize()
    return nc


def make_consts(P=128, cpg=16):
    GPS = P // cpg
    indg = np.zeros((P, GPS), np.float32)
    for p in range(P):
        indg[p, p // cpg] = 1.0
    inde = indg.T.copy()
    return indg, inde


_PROGRAM_CACHE = {}


def _get_program(C, G, N, NQ, precision="tf32"):
    key = (C, G, N, NQ, precision)
    if key not in _PROGRAM_CACHE:
        _PROGRAM_CACHE[key] = build_program(C=C, G=G, N=N, NQ=NQ,
                                            precision=precision)
    return _PROGRAM_CACHE[key]


def make_in_maps(x, gn_w, gn_b, q_w, q_b, k_w, k_b, v_w, v_b, proj_w, proj_b,
                 n_cores=8, G=32):
    """Shard full inputs into per-core input maps (weight products folded on
    host).  Per-core x is pixel-permuted so the core's query block is first;
    attention is permutation-invariant over keys so S/Z stay consistent."""
    import ml_dtypes
    bf = ml_dtypes.bfloat16
    f8 = ml_dtypes.float8_e4m3
    f = lambda a: np.ascontiguousarray(np.asarray(a, dtype=np.float32))
    x = f(x)
    b, c, h, w = x.shape
    n = h * w
    qblocks = n_cores // b
    nq = n // qblocks
    cs = c // 128
    kt = n // 128
    gps = 128 // (c // G)
    scale = np.float32(c ** -0.5)
    xf = x.reshape(b, c, n)

    def to_pcs(v):                       # [C] -> [128, CS] (c = 128*s + p)
        return np.asarray(v, np.float32).reshape(cs, 128).T

    qw, kw, vw, pw = f(q_w), f(k_w), f(v_w), f(proj_w)
    indg, inde = make_consts(cpg=c // G)
    cpk = np.zeros((128, 4 * cs + gps + 128), np.float32)
    cpk[:, 0:cs] = to_pcs(f(gn_w))
    cpk[:, cs:2 * cs] = to_pcs(f(gn_b))
    cpk[:, 2 * cs:3 * cs] = to_pcs(scale * (kw.T @ f(q_b)))
    cpk[:, 3 * cs:4 * cs] = to_pcs(pw @ f(v_b) + f(proj_b))
    cpk[:, 4 * cs:4 * cs + gps] = indg
    cpk[0:gps, 4 * cs + gps:] = inde
    common = {
        "p0t": np.ascontiguousarray((scale * (qw.T @ kw)).astype(bf)),
        "w2t": np.ascontiguousarray((pw @ vw).T.astype(bf)),
        "cpk": cpk,
    }
    in_maps = []
    for i in range(n_cores):
        bi, qi = divmod(i, qblocks)
        xb = xf[bi]
        qs, qe = qi * nq, (qi + 1) * nq
        xperm = np.concatenate([xb[:, qs:qe], xb[:, :qs], xb[:, qe:]], axis=1)
        x8 = xperm.astype(f8)
        xt8 = np.ascontiguousarray(
            x8.T.reshape(kt // 2, 2, 128, c).transpose(2, 0, 1, 3))
        in_maps.append({
            **common,
            "x": np.ascontiguousarray(xperm.astype(bf)),
            "x8": np.ascontiguousarray(x8),
            "xt8": xt8,
        })
    return in_maps, (b, c, h, w, n, nq, qblocks)


def kernel(x, gn_w, gn_b, q_w, q_b, k_w, k_b, v_w, v_b, proj_w, proj_b):
    from concourse.bass_utils import run_bass_kernel_spmd

    in_maps, (b, c, h, w, n, nq, qblocks) = make_in_maps(
        x, gn_w, gn_b, q_w, q_b, k_w, k_b, v_w, v_b, proj_w, proj_b
    )
    n_cores = 8
    nc = _get_program(C=c, G=32, N=n, NQ=nq)
    res = run_bass_kernel_spmd(nc, in_maps, list(range(n_cores))).results
    out = np.empty((b, c, n), np.float32)
    for i in range(n_cores):
        bi, qi = divmod(i, qblocks)
        out[bi, :, qi * nq:(qi + 1) * nq] = res[i]["out"]
    return out.reshape(b, c, h, w)
